# revision 10
# baseline (speedup 1.0000x reference)
"""Mistral GQA self-attention block on 8 Trainium2 NeuronCores (Bass/Tile).

Sharding: tensor-parallel over heads. Core m owns q-heads 4m..4m+3 and
kv-head m (GQA group-aligned), Wq/Wk/Wv column-sharded, Wo row-sharded.
Each core computes a full-size [B*T, H*D] partial of the output
projection; the host sums the 8 partials (the Wo row-parallel reduce).

Per-core kernel layout (feature-major, [feature, token] everywhere):
  phase 1: QKV projection  qkvT[f, t] = Wqkv_shard @ x.T, weights stationary,
           RoPE applied via a [128,128] rotation-matrix matmul + DVE combine.
  phase 2: attention per (batch, head). Scores are computed TRANSPOSED
           (st[tk, tq] = k_tile.T @ q) so the exp'd probabilities come out
           of the scalar engine already in the [tk, tq] layout the PV
           matmul needs -- no 128x128 transposes of p. The softmax
           denominator is accumulated with a ones-vector matmul and applied
           to the (8x smaller) output tile instead of to p.
  phase 3: output projection, partial[t, e] = oT.T @ WoT_shard.

Matmul dtype selectable via BASS_MM_DTYPE = f32 | f32r (default) | bf16.
f32r runs the PE at bf16 speed with ~tf32 accuracy on fp32-stored tiles.
"""

import os
import sys

import numpy as np

for _p in ("/opt/trn_rl_repo", "/root/.axon_site/_ro/trn_rl_repo"):
    if os.path.isdir(_p):
        if _p not in sys.path:
            sys.path.insert(0, _p)
        break

import ml_dtypes  # noqa: E402

B, T, H, D = 2, 1024, 32, 128
KV = 8
M = 8                 # cores
QH = H // M           # q heads per core
FT = QH + 2           # feature tiles per core: 4 q, 1 k, 1 v
CD = H * D            # contraction dim 4096
CT = CD // 128        # 32 c-tiles
BT = B * T            # 2048 tokens
NTQ = T // D          # 8 tq/tk tiles per batch
NEG = -1e9
SCALE = 1.0 / np.sqrt(D)

MM_MODE = os.environ.get("BASS_MM_DTYPE", "f32r")
assert MM_MODE in ("f32", "f32r", "bf16")


# ---------------------------------------------------------------- host prep

def _rot_matrix_T():
    """P with rot(x) = P @ x ; returns P.T as the matmul lhsT."""
    half = D // 2
    P = np.zeros((D, D), np.float32)
    for i in range(half):
        P[i, i + half] = -1.0
        P[i + half, i] = 1.0
    return np.ascontiguousarray(P.T)


def _tf32_round(a):
    """Round fp32 to the TF32 (1+8+10) representable set, RNE."""
    u = np.ascontiguousarray(a, np.float32).view(np.uint32)
    u = (u + 0x0FFF + ((u >> 13) & 1)) & np.uint32(0xFFFFE000)
    return u.view(np.float32)


def _store(a):
    """Cast a host array to the on-device storage dtype for matmul inputs."""
    if MM_MODE == "bf16":
        return np.ascontiguousarray(np.asarray(a, np.float32)).astype(
            ml_dtypes.bfloat16)
    if MM_MODE == "f32r":
        return _tf32_round(np.asarray(a, np.float32))
    return np.ascontiguousarray(np.asarray(a), np.float32)


def host_prep(stm, Wq, Wk, Wv, Wo, cos, sin, mask_b):
    """Build the 8 per-core input maps."""
    x = np.ascontiguousarray(np.asarray(stm, np.float32).reshape(BT, CD))
    xT = _store(x.T)                                     # [4096, 2048]
    cosT = np.ascontiguousarray(cos[0, :, 0, :].T, np.float32)   # [128, 1024]
    sinT = np.ascontiguousarray(sin[0, :, 0, :].T, np.float32)
    PT = _store(_rot_matrix_T())
    maskT = np.ascontiguousarray(mask_b[0, 0, :D, :D].T, np.float32)
    ones1 = _store(np.ones((D, 1), np.float32))
    ident = _store(np.eye(D, dtype=np.float32))

    in_maps = []
    for m in range(M):
        wq = Wq[m * QH * D:(m + 1) * QH * D]             # [512, 4096]
        wk = Wk[m * D:(m + 1) * D]                       # [128, 4096]
        wv = Wv[m * D:(m + 1) * D]                       # [128, 4096]
        wqkvT = _store(np.concatenate([wq, wk, wv], 0).T)  # [4096, 768]
        woT = _store(Wo[:, m * QH * D:(m + 1) * QH * D].T)  # [512, 4096]
        in_maps.append({
            "xT": xT, "wqkvT": wqkvT, "woT": woT,
            "cosT": cosT, "sinT": sinT, "PT": PT, "maskT": maskT,
            "ones1": ones1, "ident": ident,
        })
    return in_maps


# ---------------------------------------------------------------- bass prog

def _build_nc(causal=True):
    import concourse.tile as tile
    from concourse import bacc, mybir

    dt_store = {"bf16": mybir.dt.bfloat16,
                "f32r": mybir.dt.float32r,
                "f32": mybir.dt.float32}[MM_MODE]
    f32 = mybir.dt.float32

    def mc(ap):
        return ap

    nc = bacc.Bacc("TRN2", target_bir_lowering=False, debug=False)

    xT_d = nc.dram_tensor("xT", [CD, BT], dt_store, kind="ExternalInput")
    wqkvT_d = nc.dram_tensor("wqkvT", [CD, FT * D], dt_store, kind="ExternalInput")
    woT_d = nc.dram_tensor("woT", [QH * D, CD], dt_store, kind="ExternalInput")
    cosT_d = nc.dram_tensor("cosT", [D, T], f32, kind="ExternalInput")
    sinT_d = nc.dram_tensor("sinT", [D, T], f32, kind="ExternalInput")
    PT_d = nc.dram_tensor("PT", [D, D], dt_store, kind="ExternalInput")
    maskT_d = nc.dram_tensor("maskT", [D, D], f32, kind="ExternalInput")
    ones1_d = nc.dram_tensor("ones1", [D, 1], dt_store, kind="ExternalInput")
    ident_d = nc.dram_tensor("ident", [D, D], dt_store, kind="ExternalInput")
    outp_d = nc.dram_tensor("outp", [BT, CD], f32, kind="ExternalOutput")

    add = mybir.AluOpType.add
    mult = mybir.AluOpType.mult
    Exp = mybir.ActivationFunctionType.Exp

    def chunks_for_j(j):
        """Valid tq chunk ranges [(lo, hi)] for k-tile j (<=512 wide,
        psum-bank aligned ends)."""
        if not causal:
            return [(0, 512), (512, 1024)]
        w0 = D * j
        out = []
        if w0 < 512:
            out.append((w0, 512))
        out.append((max(512, w0), 1024))
        return out

    def phase1(tc, qkvT_sb):
        with tc.tile_pool(name="wqkv", bufs=1) as wpool, \
             tc.tile_pool(name="xin", bufs=4) as xpool, \
             tc.tile_pool(name="rope", bufs=3) as rpool, \
             tc.tile_pool(name="ps_qkv", bufs=1, space="PSUM") as pq, \
             tc.tile_pool(name="ps_rot", bufs=2, space="PSUM") as prot:
            w_sb = wpool.tile([128, CT, FT * D], dt_store)
            wqkvT_r = wqkvT_d.ap().rearrange("(k p) f -> p k f", p=128)
            for k in range(CT):
                nc.sync.dma_start(w_sb[:, k], wqkvT_r[:, k])
            xT_r = xT_d.ap().rearrange("(k p) t -> p k t", p=128)

            for tb in range(BT // 512):
                ps = [pq.tile([128, 512], f32, tag=f"qkv{ft}",
                              name=f"ps_qkv{ft}_{tb}")
                      for ft in range(FT)]
                for k in range(CT):
                    xk = xpool.tile([128, 512], dt_store)
                    nc.sync.dma_start(
                        xk[:], xT_r[:, k, tb * 512:(tb + 1) * 512])
                    for ft in range(FT):
                        nc.tensor.matmul(
                            ps[ft][:],
                            lhsT=mc(w_sb[:, k, ft * D:(ft + 1) * D]),
                            rhs=mc(xk[:]),
                            start=(k == 0), stop=(k == CT - 1))
                t0 = (tb % 2) * 512   # position within the rope table
                csl = cosT_sb[:, t0:t0 + 512]
                ssl = sinT_sb[:, t0:t0 + 512]
                tsl = slice(tb * 512, (tb + 1) * 512)
                for ft in range(FT):
                    dst = qkvT_sb[:, ft, tsl]
                    if ft < QH + 1:   # q heads + k head: apply RoPE
                        qraw = rpool.tile([128, 512], dt_store, tag="qraw")
                        nc.scalar.copy(qraw[:], ps[ft][:])
                        pr = prot.tile([128, 512], f32, tag="rot")
                        nc.tensor.matmul(pr[:], lhsT=mc(PT_sb[:]),
                                         rhs=mc(qraw[:]),
                                         start=True, stop=True)
                        tmp = rpool.tile([128, 512], f32, tag="rtmp")
                        nc.vector.tensor_tensor(dst, qraw[:], csl, mult)
                        nc.vector.tensor_tensor(tmp[:], pr[:], ssl, mult)
                        nc.vector.tensor_tensor(dst, dst, tmp[:], add)
                    else:             # v: plain copy
                        nc.scalar.copy(dst, ps[ft][:])

    def phase2(tc, qkvT_sb, oT_sb):
        with tc.tile_pool(name="vTp", bufs=1) as vpool, \
             tc.tile_pool(name="pT", bufs=2) as ppool, \
             tc.tile_pool(name="smx", bufs=2) as spool, \
             tc.tile_pool(name="ps_st", bufs=2, space="PSUM") as pst, \
             tc.tile_pool(name="ps_rs", bufs=1, space="PSUM") as prs, \
             tc.tile_pool(name="ps_o", bufs=1, space="PSUM") as po_pool:
            for b in range(B):
                boff = b * T
                # v -> [tk, d] tiles
                vT = vpool.tile([128, NTQ, D], dt_store, tag="vT")
                for j in range(NTQ):
                    pv = pst.tile([128, T], dt_store, tag="st")
                    nc.tensor.transpose(
                        pv[:, :D],
                        qkvT_sb[:, QH + 1, boff + j * D: boff + (j + 1) * D],
                        ident_sb[:])
                    nc.scalar.copy(vT[:, j], pv[:, :D])
                for h in range(QH):
                    qsl = qkvT_sb[:, h, boff:boff + T]
                    ksl = qkvT_sb[:, QH, boff:boff + T]
                    rs = prs.tile([128, T], f32, tag="rs")
                    pT = ppool.tile([128, NTQ, T], dt_store, tag="pT")
                    for j in range(NTQ):
                        w0 = D * j if causal else 0
                        st = pst.tile([128, T], f32, tag="st")
                        for (lo, hi) in chunks_for_j(j):
                            nc.tensor.matmul(
                                st[:, lo:hi],
                                lhsT=mc(ksl[:, j * D:(j + 1) * D]),
                                rhs=mc(qsl[:, lo:hi]),
                                start=True, stop=True)
                        if causal:
                            nc.vector.tensor_tensor(
                                st[:, w0:w0 + D], st[:, w0:w0 + D],
                                maskT_sb[:], add)
                        nc.scalar.activation(
                            pT[:, j, w0:T], st[:, w0:T], Exp,
                            scale=float(SCALE))
                        for (lo, hi) in chunks_for_j(j):
                            last_j = (min(NTQ, hi // D) - 1) if causal \
                                else NTQ - 1
                            nc.tensor.matmul(
                                rs[:1, lo:hi],
                                lhsT=mc(ones_sb[:]),
                                rhs=mc(pT[:, j, lo:hi]),
                                start=(j == 0),
                                stop=(j == last_j))
                    recip = spool.tile([1, T], f32, tag="recip")
                    nc.vector.reciprocal(recip[:], rs[:1, :])
                    bc = spool.tile([128, T], f32, tag="bc")
                    nc.gpsimd.partition_broadcast(bc[:], recip[:])
                    po = po_pool.tile([128, T], f32, tag="oT")
                    for (c0, c1) in ((0, 512), (512, 1024)):
                        js = [j for j in range(NTQ)
                              if (D * j if causal else 0) < c1]
                        for j in js:
                            lo = max(D * j, c0) if causal else c0
                            nc.tensor.matmul(
                                po[:, lo:c1],
                                lhsT=mc(vT[:, j]),
                                rhs=mc(pT[:, j, lo:c1]),
                                start=(j == 0), stop=(j == js[-1]))
                    nc.vector.tensor_tensor(
                        oT_sb[:, h, boff:boff + T], po[:], bc[:], mult)

    def phase3(tc, oT_sb):
        with tc.tile_pool(name="wo", bufs=2) as wopool, \
             tc.tile_pool(name="oout", bufs=3) as opool, \
             tc.tile_pool(name="ps_out", bufs=2, space="PSUM") as pout:
            woT_r = woT_d.ap().rearrange("(ht p) e -> p ht e", p=128)
            for eh in range(2):
                esl = slice(eh * 2048, (eh + 1) * 2048)
                w2 = wopool.tile([128, QH, 2048], dt_store, tag="w2")
                for ht in range(QH):
                    nc.sync.dma_start(w2[:, ht], woT_r[:, ht, esl])
                for tt in range(BT // 128):
                    pps = pout.tile([128, 2048], f32, tag="out")
                    for ec in range(4):
                        for h in range(QH):
                            nc.tensor.matmul(
                                pps[:, ec * 512:(ec + 1) * 512],
                                lhsT=mc(oT_sb[:, h, tt * D:(tt + 1) * D]),
                                rhs=mc(w2[:, h, ec * 512:(ec + 1) * 512]),
                                start=(h == 0), stop=(h == QH - 1))
                    ot = opool.tile([128, 2048], f32, tag="ot")
                    nc.vector.tensor_copy(ot[:, :1024], pps[:, :1024])
                    nc.scalar.copy(ot[:, 1024:], pps[:, 1024:])
                    nc.sync.dma_start(
                        outp_d.ap()[tt * 128:(tt + 1) * 128, esl],
                        ot[:])

    with tile.TileContext(nc) as tc:
        with tc.tile_pool(name="consts", bufs=1) as consts:
            cosT_sb = consts.tile([D, T], f32)
            nc.sync.dma_start(cosT_sb[:], cosT_d.ap()[:])
            sinT_sb = consts.tile([D, T], f32)
            nc.sync.dma_start(sinT_sb[:], sinT_d.ap()[:])
            PT_sb = consts.tile([D, D], dt_store)
            nc.sync.dma_start(PT_sb[:], PT_d.ap()[:])
            maskT_sb = consts.tile([D, D], f32)
            nc.sync.dma_start(maskT_sb[:], maskT_d.ap()[:])
            ones_sb = consts.tile([D, 1], dt_store)
            nc.sync.dma_start(ones_sb[:], ones1_d.ap()[:])
            ident_sb = consts.tile([D, D], dt_store)
            nc.sync.dma_start(ident_sb[:], ident_d.ap()[:])

            with tc.tile_pool(name="persist", bufs=1) as persist:
                qkvT_sb = persist.tile([128, FT, BT], dt_store)
                phase1(tc, qkvT_sb)
                with tc.tile_pool(name="persist2", bufs=1) as persist2:
                    oT_sb = persist2.tile([128, QH, BT], dt_store)
                    phase2(tc, qkvT_sb, oT_sb)
                    phase3(tc, oT_sb)

    nc.compile()
    return nc


# ---------------------------------------------------------------- runner

class _Runner:
    """Compile once, keep a no-donation jitted SPMD callable."""

    def __init__(self, causal=True):
        import jax
        from jax.sharding import Mesh, PartitionSpec
        try:
            from jax.experimental.shard_map import shard_map
        except ImportError:  # newer jax
            from jax.sharding import shard_map
        from concourse import mybir
        from concourse.bass2jax import (_bass_exec_p, install_neuronx_cc_hook,
                                        partition_id_tensor)

        self.jax = jax
        self.nc = _build_nc(causal=causal)
        nc = self.nc
        install_neuronx_cc_hook()

        partition_name = (nc.partition_id_tensor.name
                          if nc.partition_id_tensor else None)
        in_names, out_names, out_avals, zero_outs = [], [], [], []
        for alloc in nc.m.functions[0].allocations:
            if not isinstance(alloc, mybir.MemoryLocationSet):
                continue
            name = alloc.memorylocations[0].name
            if alloc.kind == "ExternalInput":
                if name != partition_name:
                    in_names.append(name)
            elif alloc.kind == "ExternalOutput":
                out_names.append(name)
                shape = tuple(alloc.tensor_shape)
                dtype = mybir.dt.np(alloc.dtype)
                out_avals.append(jax.core.ShapedArray(shape, dtype))
                zero_outs.append(np.zeros(shape, dtype))
        self.in_names, self.out_names = in_names, out_names
        self.zero_outs = zero_outs
        n_params = len(in_names)
        in_names_all = list(in_names) + list(out_names)
        if partition_name is not None:
            in_names_all.append(partition_name)

        def _body(*args):
            operands = list(args)
            if partition_name is not None:
                operands.append(partition_id_tensor())
            outs = _bass_exec_p.bind(
                *operands, out_avals=tuple(out_avals),
                in_names=tuple(in_names_all), out_names=tuple(out_names),
                lowering_input_output_aliases=(),
                sim_require_finite=True, sim_require_nnan=True, nc=nc)
            return tuple(outs)

        devices = jax.devices()[:M]
        assert len(devices) == M, f"need {M} cores, found {len(jax.devices())}"
        mesh = Mesh(np.asarray(devices), ("core",))
        in_specs = (PartitionSpec("core"),) * (n_params + len(out_names))
        out_specs = (PartitionSpec("core"),) * len(out_names)
        self.fn = jax.jit(
            shard_map(_body, mesh=mesh, in_specs=in_specs,
                      out_specs=out_specs, check_rep=False),
            keep_unused=True)

    def put_args(self, in_maps):
        jax = self.jax
        concat_in = [np.concatenate([in_maps[c][nm] for c in range(M)], axis=0)
                     for nm in self.in_names]
        concat_zeros = [np.zeros((M * z.shape[0], *z.shape[1:]), z.dtype)
                        for z in self.zero_outs]
        return [jax.device_put(x) for x in concat_in + concat_zeros]

    def run(self, args):
        outs = self.fn(*args)
        self.jax.block_until_ready(outs)
        return outs

    def gather(self, outs):
        """Sum the 8 partials of 'outp' -> full [B,T,H,D] output."""
        i = self.out_names.index("outp")
        arr = np.asarray(outs[i]).reshape(M, BT, CD)
        return arr.sum(0, dtype=np.float32).reshape(B, T, H, D)


_RUNNERS = {}


def _get_runner(causal=True):
    if causal not in _RUNNERS:
        _RUNNERS[causal] = _Runner(causal=causal)
    return _RUNNERS[causal]


def _mask_kind(mask_w, mask_b):
    tril = np.tril(np.ones((T, T), np.float32))
    if (np.array_equal(mask_w[0, 0], tril)
            and np.allclose(mask_b[0, 0], (1.0 - tril) * NEG)):
        return "causal"
    if (mask_w == 1.0).all() and (mask_b == 0.0).all():
        return "allpass"
    return "other"


def _numpy_fallback(stm, Wq, Wk, Wv, Wo, cos, sin, mask_w, mask_b):
    x = stm.reshape(B, T, H * D).astype(np.float32)
    q = (x @ Wq.T).reshape(B, T, H, D)
    k = (x @ Wk.T).reshape(B, T, KV, D)
    v = (x @ Wv.T).reshape(B, T, KV, D)
    k = np.repeat(k, H // KV, axis=2)
    v = np.repeat(v, H // KV, axis=2)

    def rope(t):
        half = D // 2
        t2 = np.concatenate([-t[..., half:], t[..., :half]], -1)
        return t * cos + t2 * sin

    q, k = rope(q), rope(k)
    attn = np.einsum("bqhd,bkhd->bhqk", q, k).astype(np.float32) * SCALE
    attn = attn * mask_w + mask_b
    attn = attn - attn.max(-1, keepdims=True)
    attn = np.exp(attn)
    attn = attn / attn.sum(-1, keepdims=True)
    o = np.einsum("bhqk,bkhd->bqhd", attn, v).astype(np.float32)
    return (o.reshape(B, T, H * D) @ Wo.T).reshape(B, T, H, D)


def kernel(stm, Wq, Wk, Wv, Wo, cos, sin, mask_w, mask_b):
    stm = np.asarray(stm, np.float32)
    Wq, Wk, Wv, Wo = (np.asarray(a, np.float32) for a in (Wq, Wk, Wv, Wo))
    cos, sin = np.asarray(cos, np.float32), np.asarray(sin, np.float32)
    mask_w, mask_b = (np.asarray(a, np.float32) for a in (mask_w, mask_b))

    kind = _mask_kind(mask_w, mask_b)
    if kind == "other":
        return _numpy_fallback(stm, Wq, Wk, Wv, Wo, cos, sin, mask_w, mask_b)

    runner = _get_runner(causal=(kind == "causal"))
    in_maps = host_prep(stm, Wq, Wk, Wv, Wo, cos, sin, mask_b)
    args = runner.put_args(in_maps)
    outs = runner.run(args)
    return runner.gather(outs)


# revision 11
# speedup vs baseline: 95.3745x; 95.3745x over previous
"""Mistral GQA self-attention block on 8 Trainium2 NeuronCores (Bass/Tile).

Sharding: tensor-parallel over heads. Core m owns q-heads 4m..4m+3 and
kv-head m (GQA group-aligned), Wq/Wk/Wv column-sharded, Wo row-sharded.
Each core computes a full-size [B*T, H*D] partial of the output
projection; the host sums the 8 partials (the Wo row-parallel reduce).

Per-core kernel layout (feature-major, [feature, token] everywhere):
  phase 1: QKV projection  qkvT[f, t] = Wqkv_shard @ x.T, weights stationary,
           RoPE applied via a [128,128] rotation-matrix matmul + DVE combine.
  phase 2: attention per (batch, head). Scores are computed TRANSPOSED
           (st[tk, tq] = k_tile.T @ q) so the exp'd probabilities come out
           of the scalar engine already in the [tk, tq] layout the PV
           matmul needs -- no 128x128 transposes of p. The softmax
           denominator is accumulated with a ones-vector matmul and applied
           to the (8x smaller) output tile instead of to p.
  phase 3: output projection, partial[t, e] = oT.T @ WoT_shard.

Matmul dtype selectable via BASS_MM_DTYPE = f32 | f32r (default) | bf16.
f32r runs the PE at bf16 speed with ~tf32 accuracy on fp32-stored tiles.
"""

import os
import sys

import numpy as np

for _p in ("/opt/trn_rl_repo", "/root/.axon_site/_ro/trn_rl_repo"):
    if os.path.isdir(_p):
        if _p not in sys.path:
            sys.path.insert(0, _p)
        break

import ml_dtypes  # noqa: E402

B, T, H, D = 2, 1024, 32, 128
KV = 8
M = 8                 # cores
QH = H // M           # q heads per core
FT = QH + 2           # feature tiles per core: 4 q, 1 k, 1 v
CD = H * D            # contraction dim 4096
CT = CD // 128        # 32 c-tiles
BT = B * T            # 2048 tokens
NTQ = T // D          # 8 tq/tk tiles per batch
NEG = -1e9
SCALE = 1.0 / np.sqrt(D)

MM_MODE = os.environ.get("BASS_MM_DTYPE", "f32r")
assert MM_MODE in ("f32", "f32r", "bf16")


# ---------------------------------------------------------------- host prep

def _rot_matrix_T():
    """P with rot(x) = P @ x ; returns P.T as the matmul lhsT."""
    half = D // 2
    P = np.zeros((D, D), np.float32)
    for i in range(half):
        P[i, i + half] = -1.0
        P[i + half, i] = 1.0
    return np.ascontiguousarray(P.T)


def _tf32_round(a):
    """Round fp32 to the TF32 (1+8+10) representable set, RNE."""
    u = np.ascontiguousarray(a, np.float32).view(np.uint32)
    u = (u + 0x0FFF + ((u >> 13) & 1)) & np.uint32(0xFFFFE000)
    return u.view(np.float32)


def _store(a):
    """Cast a host array to the on-device storage dtype for matmul inputs."""
    if MM_MODE == "bf16":
        return np.ascontiguousarray(np.asarray(a, np.float32)).astype(
            ml_dtypes.bfloat16)
    if MM_MODE == "f32r":
        return _tf32_round(np.asarray(a, np.float32))
    return np.ascontiguousarray(np.asarray(a), np.float32)


def host_prep(stm, Wq, Wk, Wv, Wo, cos, sin, mask_b):
    """Build the 8 per-core input maps."""
    x = np.ascontiguousarray(np.asarray(stm, np.float32).reshape(BT, CD))
    xT = _store(x.T)                                     # [4096, 2048]
    cosT = np.ascontiguousarray(cos[0, :, 0, :].T, np.float32)   # [128, 1024]
    sinT = np.ascontiguousarray(sin[0, :, 0, :].T, np.float32)
    PT = _store(_rot_matrix_T())
    maskT = np.ascontiguousarray(mask_b[0, 0, :D, :D].T, np.float32)
    ones1 = _store(np.ones((D, 1), np.float32))
    ident = _store(np.eye(D, dtype=np.float32))

    in_maps = []
    for m in range(M):
        wq = Wq[m * QH * D:(m + 1) * QH * D]             # [512, 4096]
        wk = Wk[m * D:(m + 1) * D]                       # [128, 4096]
        wv = Wv[m * D:(m + 1) * D]                       # [128, 4096]
        wqkvT = _store(np.concatenate([wq, wk, wv], 0).T)  # [4096, 768]
        woT = _store(Wo[:, m * QH * D:(m + 1) * QH * D].T)  # [512, 4096]
        in_maps.append({
            "xT": xT, "wqkvT": wqkvT, "woT": woT,
            "cosT": cosT, "sinT": sinT, "PT": PT, "maskT": maskT,
            "ones1": ones1, "ident": ident,
        })
    return in_maps


# ---------------------------------------------------------------- bass prog

def _build_nc(causal=True):
    import concourse.tile as tile
    from concourse import bacc, mybir

    dt_store = {"bf16": mybir.dt.bfloat16,
                "f32r": mybir.dt.float32r,
                "f32": mybir.dt.float32}[MM_MODE]
    f32 = mybir.dt.float32

    def mc(ap):
        return ap

    nc = bacc.Bacc("TRN2", target_bir_lowering=False, debug=False)

    xT_d = nc.dram_tensor("xT", [CD, BT], dt_store, kind="ExternalInput")
    wqkvT_d = nc.dram_tensor("wqkvT", [CD, FT * D], dt_store, kind="ExternalInput")
    woT_d = nc.dram_tensor("woT", [QH * D, CD], dt_store, kind="ExternalInput")
    cosT_d = nc.dram_tensor("cosT", [D, T], f32, kind="ExternalInput")
    sinT_d = nc.dram_tensor("sinT", [D, T], f32, kind="ExternalInput")
    PT_d = nc.dram_tensor("PT", [D, D], dt_store, kind="ExternalInput")
    maskT_d = nc.dram_tensor("maskT", [D, D], f32, kind="ExternalInput")
    ones1_d = nc.dram_tensor("ones1", [D, 1], dt_store, kind="ExternalInput")
    ident_d = nc.dram_tensor("ident", [D, D], dt_store, kind="ExternalInput")
    outp_d = nc.dram_tensor("outp", [BT, CD], f32, kind="ExternalOutput")

    add = mybir.AluOpType.add
    mult = mybir.AluOpType.mult
    Exp = mybir.ActivationFunctionType.Exp

    def chunks_for_j(j):
        """Valid tq chunk ranges [(lo, hi)] for k-tile j (<=512 wide,
        psum-bank aligned ends)."""
        if not causal:
            return [(0, 512), (512, 1024)]
        w0 = D * j
        out = []
        if w0 < 512:
            out.append((w0, 512))
        out.append((max(512, w0), 1024))
        return out

    def phase1(tc, qkvT_sb):
        with tc.tile_pool(name="wqkv", bufs=1) as wpool, \
             tc.tile_pool(name="xin", bufs=4) as xpool, \
             tc.tile_pool(name="rope", bufs=3) as rpool, \
             tc.tile_pool(name="ps_qkv", bufs=1, space="PSUM") as pq, \
             tc.tile_pool(name="ps_rot", bufs=2, space="PSUM") as prot:
            w_sb = wpool.tile([128, CT, FT * D], dt_store)
            wqkvT_r = wqkvT_d.ap().rearrange("(k p) f -> p k f", p=128)
            for k in range(CT):
                nc.sync.dma_start(w_sb[:, k], wqkvT_r[:, k])
            xT_r = xT_d.ap().rearrange("(k p) t -> p k t", p=128)

            for tb in range(BT // 512):
                ps = [pq.tile([128, 512], f32, tag=f"qkv{ft}",
                              name=f"ps_qkv{ft}_{tb}")
                      for ft in range(FT)]
                for k in range(CT):
                    xk = xpool.tile([128, 512], dt_store)
                    nc.sync.dma_start(
                        xk[:], xT_r[:, k, tb * 512:(tb + 1) * 512])
                    for ft in range(FT):
                        nc.tensor.matmul(
                            ps[ft][:],
                            lhsT=mc(w_sb[:, k, ft * D:(ft + 1) * D]),
                            rhs=mc(xk[:]),
                            start=(k == 0), stop=(k == CT - 1))
                t0 = (tb % 2) * 512   # position within the rope table
                csl = cosT_sb[:, t0:t0 + 512]
                ssl = sinT_sb[:, t0:t0 + 512]
                tsl = slice(tb * 512, (tb + 1) * 512)
                for ft in range(FT):
                    dst = qkvT_sb[:, ft, tsl]
                    if ft < QH + 1:   # q heads + k head: apply RoPE
                        qraw = rpool.tile([128, 512], dt_store, tag="qraw")
                        nc.scalar.copy(qraw[:], ps[ft][:])
                        pr = prot.tile([128, 512], f32, tag="rot")
                        nc.tensor.matmul(pr[:], lhsT=mc(PT_sb[:]),
                                         rhs=mc(qraw[:]),
                                         start=True, stop=True)
                        tmp = rpool.tile([128, 512], f32, tag="rtmp")
                        nc.vector.tensor_tensor(dst, qraw[:], csl, mult)
                        nc.vector.tensor_tensor(tmp[:], pr[:], ssl, mult)
                        nc.vector.tensor_tensor(dst, dst, tmp[:], add)
                    else:             # v: plain copy
                        nc.scalar.copy(dst, ps[ft][:])

    def phase2(tc, qkvT_sb, oT_sb):
        with tc.tile_pool(name="vTp", bufs=1) as vpool, \
             tc.tile_pool(name="pT", bufs=2) as ppool, \
             tc.tile_pool(name="smx", bufs=2) as spool, \
             tc.tile_pool(name="ps_st", bufs=2, space="PSUM") as pst, \
             tc.tile_pool(name="ps_rs", bufs=1, space="PSUM") as prs, \
             tc.tile_pool(name="ps_o", bufs=1, space="PSUM") as po_pool:
            for b in range(B):
                boff = b * T
                # v -> [tk, d] tiles
                vT = vpool.tile([128, NTQ, D], dt_store, tag="vT")
                for j in range(NTQ):
                    pv = pst.tile([128, T], dt_store, tag="st")
                    nc.tensor.transpose(
                        pv[:, :D],
                        qkvT_sb[:, QH + 1, boff + j * D: boff + (j + 1) * D],
                        ident_sb[:])
                    nc.scalar.copy(vT[:, j], pv[:, :D])
                for h in range(QH):
                    qsl = qkvT_sb[:, h, boff:boff + T]
                    ksl = qkvT_sb[:, QH, boff:boff + T]
                    rs = prs.tile([128, T], f32, tag="rs")
                    pT = ppool.tile([128, NTQ, T], dt_store, tag="pT")
                    for j in range(NTQ):
                        w0 = D * j if causal else 0
                        st = pst.tile([128, T], f32, tag="st")
                        for (lo, hi) in chunks_for_j(j):
                            nc.tensor.matmul(
                                st[:, lo:hi],
                                lhsT=mc(ksl[:, j * D:(j + 1) * D]),
                                rhs=mc(qsl[:, lo:hi]),
                                start=True, stop=True)
                        if causal:
                            nc.vector.tensor_tensor(
                                st[:, w0:w0 + D], st[:, w0:w0 + D],
                                maskT_sb[:], add)
                        nc.scalar.activation(
                            pT[:, j, w0:T], st[:, w0:T], Exp,
                            scale=float(SCALE))
                        for (lo, hi) in chunks_for_j(j):
                            last_j = (min(NTQ, hi // D) - 1) if causal \
                                else NTQ - 1
                            nc.tensor.matmul(
                                rs[:1, lo:hi],
                                lhsT=mc(ones_sb[:]),
                                rhs=mc(pT[:, j, lo:hi]),
                                start=(j == 0),
                                stop=(j == last_j))
                    recip = spool.tile([1, T], f32, tag="recip")
                    nc.vector.reciprocal(recip[:], rs[:1, :])
                    bc = spool.tile([128, T], f32, tag="bc")
                    nc.gpsimd.partition_broadcast(bc[:], recip[:])
                    po = po_pool.tile([128, T], f32, tag="oT")
                    for (c0, c1) in ((0, 512), (512, 1024)):
                        js = [j for j in range(NTQ)
                              if (D * j if causal else 0) < c1]
                        for j in js:
                            lo = max(D * j, c0) if causal else c0
                            nc.tensor.matmul(
                                po[:, lo:c1],
                                lhsT=mc(vT[:, j]),
                                rhs=mc(pT[:, j, lo:c1]),
                                start=(j == 0), stop=(j == js[-1]))
                    nc.vector.tensor_tensor(
                        oT_sb[:, h, boff:boff + T], po[:], bc[:], mult)

    def phase3(tc, oT_sb):
        with tc.tile_pool(name="wo", bufs=2) as wopool, \
             tc.tile_pool(name="oout", bufs=3) as opool, \
             tc.tile_pool(name="ps_out", bufs=2, space="PSUM") as pout:
            woT_r = woT_d.ap().rearrange("(ht p) e -> p ht e", p=128)
            for eh in range(2):
                esl = slice(eh * 2048, (eh + 1) * 2048)
                w2 = wopool.tile([128, QH, 2048], dt_store, tag="w2")
                for ht in range(QH):
                    nc.sync.dma_start(w2[:, ht], woT_r[:, ht, esl])
                for tt in range(BT // 128):
                    pps = pout.tile([128, 2048], f32, tag="out")
                    for ec in range(4):
                        for h in range(QH):
                            nc.tensor.matmul(
                                pps[:, ec * 512:(ec + 1) * 512],
                                lhsT=mc(oT_sb[:, h, tt * D:(tt + 1) * D]),
                                rhs=mc(w2[:, h, ec * 512:(ec + 1) * 512]),
                                start=(h == 0), stop=(h == QH - 1))
                    ot = opool.tile([128, 2048], f32, tag="ot")
                    nc.vector.tensor_copy(ot[:, :1024], pps[:, :1024])
                    nc.scalar.copy(ot[:, 1024:], pps[:, 1024:])
                    nc.sync.dma_start(
                        outp_d.ap()[tt * 128:(tt + 1) * 128, esl],
                        ot[:])

    with tile.TileContext(nc) as tc:
        with tc.tile_pool(name="consts", bufs=1) as consts:
            cosT_sb = consts.tile([D, T], f32)
            nc.sync.dma_start(cosT_sb[:], cosT_d.ap()[:])
            sinT_sb = consts.tile([D, T], f32)
            nc.sync.dma_start(sinT_sb[:], sinT_d.ap()[:])
            PT_sb = consts.tile([D, D], dt_store)
            nc.sync.dma_start(PT_sb[:], PT_d.ap()[:])
            maskT_sb = consts.tile([D, D], f32)
            nc.sync.dma_start(maskT_sb[:], maskT_d.ap()[:])
            ones_sb = consts.tile([D, 1], dt_store)
            nc.sync.dma_start(ones_sb[:], ones1_d.ap()[:])
            ident_sb = consts.tile([D, D], dt_store)
            nc.sync.dma_start(ident_sb[:], ident_d.ap()[:])

            with tc.tile_pool(name="persist", bufs=1) as persist:
                qkvT_sb = persist.tile([128, FT, BT], dt_store)
                phase1(tc, qkvT_sb)
                with tc.tile_pool(name="persist2", bufs=1) as persist2:
                    oT_sb = persist2.tile([128, QH, BT], dt_store)
                    phase2(tc, qkvT_sb, oT_sb)
                    phase3(tc, oT_sb)

    nc.compile()
    return nc


# ---------------------------------------------------------------- runner

class _Runner:
    """Compile once, keep a no-donation jitted SPMD callable."""

    def __init__(self, causal=True):
        import jax
        from jax.sharding import Mesh, PartitionSpec
        try:
            from jax.experimental.shard_map import shard_map
        except ImportError:  # newer jax
            from jax.sharding import shard_map
        from concourse import mybir
        from concourse.bass2jax import (_bass_exec_p, install_neuronx_cc_hook,
                                        partition_id_tensor)

        self.jax = jax
        self.nc = _build_nc(causal=causal)
        nc = self.nc
        install_neuronx_cc_hook()

        partition_name = (nc.partition_id_tensor.name
                          if nc.partition_id_tensor else None)
        in_names, out_names, out_avals, zero_outs = [], [], [], []
        for alloc in nc.m.functions[0].allocations:
            if not isinstance(alloc, mybir.MemoryLocationSet):
                continue
            name = alloc.memorylocations[0].name
            if alloc.kind == "ExternalInput":
                if name != partition_name:
                    in_names.append(name)
            elif alloc.kind == "ExternalOutput":
                out_names.append(name)
                shape = tuple(alloc.tensor_shape)
                dtype = mybir.dt.np(alloc.dtype)
                out_avals.append(jax.core.ShapedArray(shape, dtype))
                zero_outs.append(np.zeros(shape, dtype))
        self.in_names, self.out_names = in_names, out_names
        self.zero_outs = zero_outs
        n_params = len(in_names)
        in_names_all = list(in_names) + list(out_names)
        if partition_name is not None:
            in_names_all.append(partition_name)

        def _body(*args):
            operands = list(args)
            if partition_name is not None:
                operands.append(partition_id_tensor())
            outs = _bass_exec_p.bind(
                *operands, out_avals=tuple(out_avals),
                in_names=tuple(in_names_all), out_names=tuple(out_names),
                lowering_input_output_aliases=(),
                sim_require_finite=True, sim_require_nnan=True, nc=nc)
            return tuple(outs)

        devices = jax.devices()[:M]
        assert len(devices) == M, f"need {M} cores, found {len(jax.devices())}"
        mesh = Mesh(np.asarray(devices), ("core",))
        self.mesh = mesh
        in_specs = (PartitionSpec("core"),) * (n_params + len(out_names))
        out_specs = (PartitionSpec("core"),) * len(out_names)
        # Donate the output-shaped args: the NEFF fully overwrites every
        # output tensor, so we ping-pong the previous call's outputs in as
        # the next call's donated output buffers.
        donate = tuple(range(n_params, n_params + len(out_names)))
        self.fn = jax.jit(
            shard_map(_body, mesh=mesh, in_specs=in_specs,
                      out_specs=out_specs, check_rep=False),
            keep_unused=True, donate_argnums=donate)

    def put_args(self, in_maps):
        jax = self.jax
        from jax.sharding import NamedSharding, PartitionSpec
        sh = NamedSharding(self.mesh, PartitionSpec("core"))
        concat_in = [np.concatenate([in_maps[c][nm] for c in range(M)], axis=0)
                     for nm in self.in_names]
        args = [jax.device_put(x, sh) for x in concat_in]
        self._outbufs = [
            jax.device_put(np.zeros((M * z.shape[0], *z.shape[1:]), z.dtype), sh)
            for z in self.zero_outs]
        return args

    def run(self, args):
        outs = self.fn(*args, *self._outbufs)
        self.jax.block_until_ready(outs)
        self._outbufs = list(outs)   # donated ping-pong
        return outs

    def gather(self, outs):
        """Sum the 8 partials of 'outp' -> full [B,T,H,D] output."""
        i = self.out_names.index("outp")
        arr = np.asarray(outs[i]).reshape(M, BT, CD)
        return arr.sum(0, dtype=np.float32).reshape(B, T, H, D)


_RUNNERS = {}


def _get_runner(causal=True):
    if causal not in _RUNNERS:
        _RUNNERS[causal] = _Runner(causal=causal)
    return _RUNNERS[causal]


def _mask_kind(mask_w, mask_b):
    tril = np.tril(np.ones((T, T), np.float32))
    if (np.array_equal(mask_w[0, 0], tril)
            and np.allclose(mask_b[0, 0], (1.0 - tril) * NEG)):
        return "causal"
    if (mask_w == 1.0).all() and (mask_b == 0.0).all():
        return "allpass"
    return "other"


def _numpy_fallback(stm, Wq, Wk, Wv, Wo, cos, sin, mask_w, mask_b):
    x = stm.reshape(B, T, H * D).astype(np.float32)
    q = (x @ Wq.T).reshape(B, T, H, D)
    k = (x @ Wk.T).reshape(B, T, KV, D)
    v = (x @ Wv.T).reshape(B, T, KV, D)
    k = np.repeat(k, H // KV, axis=2)
    v = np.repeat(v, H // KV, axis=2)

    def rope(t):
        half = D // 2
        t2 = np.concatenate([-t[..., half:], t[..., :half]], -1)
        return t * cos + t2 * sin

    q, k = rope(q), rope(k)
    attn = np.einsum("bqhd,bkhd->bhqk", q, k).astype(np.float32) * SCALE
    attn = attn * mask_w + mask_b
    attn = attn - attn.max(-1, keepdims=True)
    attn = np.exp(attn)
    attn = attn / attn.sum(-1, keepdims=True)
    o = np.einsum("bhqk,bkhd->bqhd", attn, v).astype(np.float32)
    return (o.reshape(B, T, H * D) @ Wo.T).reshape(B, T, H, D)


def kernel(stm, Wq, Wk, Wv, Wo, cos, sin, mask_w, mask_b):
    stm = np.asarray(stm, np.float32)
    Wq, Wk, Wv, Wo = (np.asarray(a, np.float32) for a in (Wq, Wk, Wv, Wo))
    cos, sin = np.asarray(cos, np.float32), np.asarray(sin, np.float32)
    mask_w, mask_b = (np.asarray(a, np.float32) for a in (mask_w, mask_b))

    kind = _mask_kind(mask_w, mask_b)
    if kind == "other":
        return _numpy_fallback(stm, Wq, Wk, Wv, Wo, cos, sin, mask_w, mask_b)

    runner = _get_runner(causal=(kind == "causal"))
    in_maps = host_prep(stm, Wq, Wk, Wv, Wo, cos, sin, mask_b)
    args = runner.put_args(in_maps)
    outs = runner.run(args)
    return runner.gather(outs)


# revision 17
# speedup vs baseline: 98.2071x; 1.0297x over previous
"""Mistral GQA self-attention block on 8 Trainium2 NeuronCores (Bass/Tile).

Sharding: tensor-parallel over heads. Core m owns q-heads 4m..4m+3 and
kv-head m (GQA group-aligned), Wq/Wk/Wv column-sharded, Wo row-sharded.
Each core computes a full-size [B*T, H*D] partial of the output
projection; the host sums the 8 partials (the Wo row-parallel reduce).

Per-core kernel layout (feature-major, [feature, token] everywhere):
  phase 1: QKV projection  qkvT[f, t] = Wqkv_shard @ x.T, weights stationary,
           RoPE applied via a [128,128] rotation-matrix matmul + DVE combine.
  phase 2: attention per (batch, head). Scores are computed TRANSPOSED
           (st[tk, tq] = k_tile.T @ q) so the exp'd probabilities come out
           of the scalar engine already in the [tk, tq] layout the PV
           matmul needs -- no 128x128 transposes of p. The softmax
           denominator is accumulated with a ones-vector matmul and applied
           to the (8x smaller) output tile instead of to p.
  phase 3: output projection, partial[t, e] = oT.T @ WoT_shard.

Matmul dtype selectable via BASS_MM_DTYPE = f32 | f32r (default) | bf16.
f32r runs the PE at bf16 speed with ~tf32 accuracy on fp32-stored tiles.
"""

import os
import sys

import numpy as np

for _p in ("/opt/trn_rl_repo", "/root/.axon_site/_ro/trn_rl_repo"):
    if os.path.isdir(_p):
        if _p not in sys.path:
            sys.path.insert(0, _p)
        break

import ml_dtypes  # noqa: E402

B, T, H, D = 2, 1024, 32, 128
KV = 8
M = 8                 # cores
QH = H // M           # q heads per core
FT = QH + 2           # feature tiles per core: 4 q, 1 k, 1 v
CD = H * D            # contraction dim 4096
CT = CD // 128        # 32 c-tiles
BT = B * T            # 2048 tokens
NTQ = T // D          # 8 tq/tk tiles per batch
NEG = -1e9
SCALE = 1.0 / np.sqrt(D)

MM_MODE = os.environ.get("BASS_MM_DTYPE", "f32r")
assert MM_MODE in ("f32", "f32r", "bf16")


# ---------------------------------------------------------------- host prep

def _rot_matrix_T():
    """P with rot(x) = P @ x ; returns P.T as the matmul lhsT."""
    half = D // 2
    P = np.zeros((D, D), np.float32)
    for i in range(half):
        P[i, i + half] = -1.0
        P[i + half, i] = 1.0
    return np.ascontiguousarray(P.T)


def _tf32_round(a):
    """Round fp32 to the TF32 (1+8+10) representable set, RNE."""
    u = np.ascontiguousarray(a, np.float32).view(np.uint32)
    u = (u + 0x0FFF + ((u >> 13) & 1)) & np.uint32(0xFFFFE000)
    return u.view(np.float32)


def _store(a):
    """Cast a host array to the on-device storage dtype for matmul inputs."""
    if MM_MODE == "bf16":
        return np.ascontiguousarray(np.asarray(a, np.float32)).astype(
            ml_dtypes.bfloat16)
    if MM_MODE == "f32r":
        return _tf32_round(np.asarray(a, np.float32))
    return np.ascontiguousarray(np.asarray(a), np.float32)


def host_prep(stm, Wq, Wk, Wv, Wo, cos, sin, mask_b):
    """Build the 8 per-core input maps."""
    x = np.ascontiguousarray(np.asarray(stm, np.float32).reshape(BT, CD))
    xT = _store(x.T)                                     # [4096, 2048]
    cosT = np.ascontiguousarray(cos[0, :, 0, :].T, np.float32)   # [128, 1024]
    sinT = np.ascontiguousarray(sin[0, :, 0, :].T, np.float32)
    PT = _store(_rot_matrix_T())
    maskT = np.ascontiguousarray(mask_b[0, 0, :D, :D].T, np.float32)
    # 0/1 validity of the diagonal block in [tk, tq] layout (tk <= tq)
    tril01 = _store((maskT == 0.0).astype(np.float32))
    ones1 = _store(np.ones((D, 1), np.float32))
    ident = _store(np.eye(D, dtype=np.float32))

    in_maps = []
    for m in range(M):
        wq = Wq[m * QH * D:(m + 1) * QH * D]             # [512, 4096]
        wk = Wk[m * D:(m + 1) * D]                       # [128, 4096]
        wv = Wv[m * D:(m + 1) * D]                       # [128, 4096]
        wqkvT = _store(np.concatenate([wq, wk, wv], 0).T)  # [4096, 768]
        woT = _store(Wo[:, m * QH * D:(m + 1) * QH * D].T)  # [512, 4096]
        in_maps.append({
            "xT": xT, "wqkvT": wqkvT, "woT": woT,
            "cosT": cosT, "sinT": sinT, "PT": PT, "maskT": maskT,
            "tril01": tril01, "ones1": ones1, "ident": ident,
        })
    return in_maps


# ---------------------------------------------------------------- bass prog

def _build_nc(causal=True):
    import concourse.tile as tile
    from concourse import bacc, mybir

    dt_store = {"bf16": mybir.dt.bfloat16,
                "f32r": mybir.dt.float32r,
                "f32": mybir.dt.float32}[MM_MODE]
    f32 = mybir.dt.float32

    def mc(ap):
        return ap

    nc = bacc.Bacc("TRN2", target_bir_lowering=False, debug=False)

    xT_d = nc.dram_tensor("xT", [CD, BT], dt_store, kind="ExternalInput")
    wqkvT_d = nc.dram_tensor("wqkvT", [CD, FT * D], dt_store, kind="ExternalInput")
    woT_d = nc.dram_tensor("woT", [QH * D, CD], dt_store, kind="ExternalInput")
    cosT_d = nc.dram_tensor("cosT", [D, T], f32, kind="ExternalInput")
    sinT_d = nc.dram_tensor("sinT", [D, T], f32, kind="ExternalInput")
    PT_d = nc.dram_tensor("PT", [D, D], dt_store, kind="ExternalInput")
    maskT_d = nc.dram_tensor("maskT", [D, D], f32, kind="ExternalInput")
    tril01_d = nc.dram_tensor("tril01", [D, D], dt_store, kind="ExternalInput")
    ones1_d = nc.dram_tensor("ones1", [D, 1], dt_store, kind="ExternalInput")
    ident_d = nc.dram_tensor("ident", [D, D], dt_store, kind="ExternalInput")
    outp_d = nc.dram_tensor("outp", [BT, CD], f32, kind="ExternalOutput")

    add = mybir.AluOpType.add
    mult = mybir.AluOpType.mult
    Exp = mybir.ActivationFunctionType.Exp

    def chunks_for_j(j):
        """Valid tq chunk ranges [(lo, hi)] for k-tile j (<=512 wide,
        psum-bank aligned ends)."""
        if not causal:
            return [(0, 512), (512, 1024)]
        w0 = D * j
        out = []
        if w0 < 512:
            out.append((w0, 512))
        out.append((max(512, w0), 1024))
        return out

    def phase1(tc, qkvT_sb, issue_consts):
        with tc.tile_pool(name="wqkv", bufs=1) as wpool, \
             tc.tile_pool(name="xin", bufs=4) as xpool, \
             tc.tile_pool(name="rope", bufs=3) as rpool, \
             tc.tile_pool(name="ps_qkv", bufs=1, space="PSUM") as pq, \
             tc.tile_pool(name="ps_rot", bufs=2, space="PSUM") as prot:
            wqkvT_r = wqkvT_d.ap().rearrange("(k p) f -> p k f", p=128)
            w_sb = [wpool.tile([128, FT * D], dt_store, tag=f"w{k}",
                               name=f"w_{k}") for k in range(CT)]
            xT_r = xT_d.ap().rearrange("(k p) t -> p k t", p=128)

            for tb in range(BT // 512):
                ps = [pq.tile([128, 512], f32, tag=f"qkv{ft}",
                              name=f"ps_qkv{ft}_{tb}")
                      for ft in range(FT)]
                for k in range(CT):
                    if tb == 0:
                        nc.sync.dma_start(w_sb[k][:], wqkvT_r[:, k])
                    xk = xpool.tile([128, 512], dt_store)
                    nc.sync.dma_start(
                        xk[:], xT_r[:, k, tb * 512:(tb + 1) * 512])
                    if tb == 0 and k == 1:
                        issue_consts()
                    for ft in range(FT):
                        nc.tensor.matmul(
                            ps[ft][:],
                            lhsT=mc(w_sb[k][:, ft * D:(ft + 1) * D]),
                            rhs=mc(xk[:]),
                            start=(k == 0), stop=(k == CT - 1))
                t0 = (tb % 2) * 512   # position within the rope table
                csl = cosT_sb[:, t0:t0 + 512]
                ssl = sinT_sb[:, t0:t0 + 512]
                bb, tsl = tb // 2, slice((tb % 2) * 512, (tb % 2) * 512 + 512)
                for ft in range(FT):
                    dst = qkvT_sb[ft][bb][:, tsl]
                    if ft < QH + 1:   # q heads + k head: apply RoPE
                        qraw = rpool.tile([128, 512], dt_store, tag="qraw")
                        nc.scalar.copy(qraw[:], ps[ft][:])
                        pr = prot.tile([128, 512], f32, tag="rot")
                        nc.tensor.matmul(pr[:], lhsT=mc(PT_sb[:]),
                                         rhs=mc(qraw[:]),
                                         start=True, stop=True)
                        tmp = rpool.tile([128, 512], f32, tag="rtmp")
                        nc.vector.tensor_tensor(dst, qraw[:], csl, mult)
                        nc.vector.tensor_tensor(tmp[:], pr[:], ssl, mult)
                        nc.vector.tensor_tensor(dst, dst, tmp[:], add)
                    else:             # v: plain copy
                        nc.scalar.copy(dst, ps[ft][:])

    def phase2(tc, qkvT_sb, oT_sb):
        with tc.tile_pool(name="vTp", bufs=1) as vpool, \
             tc.tile_pool(name="pT", bufs=2) as ppool, \
             tc.tile_pool(name="smx", bufs=2) as spool, \
             tc.tile_pool(name="ps_st", bufs=2, space="PSUM") as pst, \
             tc.tile_pool(name="ps_rs", bufs=1, space="PSUM") as prs, \
             tc.tile_pool(name="ps_o", bufs=1, space="PSUM") as po_pool:
            for b in range(B):
                boff = b * T
                # v -> [tk, d] tiles
                vT = vpool.tile([128, NTQ, D], dt_store, tag="vT")
                for j in range(NTQ):
                    pv = pst.tile([128, T], dt_store, tag="st")
                    nc.tensor.transpose(
                        pv[:, :D],
                        qkvT_sb[QH + 1][b][:, j * D:(j + 1) * D],
                        ident_sb[:])
                    nc.scalar.copy(vT[:, j], pv[:, :D])
                for h in range(QH):
                    qsl = qkvT_sb[h][b][:]
                    ksl = qkvT_sb[QH][b][:]
                    rs = prs.tile([128, T], f32, tag="rs")
                    pT = ppool.tile([128, NTQ, T], dt_store, tag="pT")
                    for j in range(NTQ):
                        w0 = D * j if causal else 0
                        st = pst.tile([128, T], f32, tag="st")
                        for (lo, hi) in chunks_for_j(j):
                            nc.tensor.matmul(
                                st[:, lo:hi],
                                lhsT=mc(ksl[:, j * D:(j + 1) * D]),
                                rhs=mc(qsl[:, lo:hi]),
                                start=True, stop=True)
                        nc.scalar.activation(
                            pT[:, j, w0:T], st[:, w0:T], Exp,
                            scale=float(SCALE))
                        if causal:
                            nc.gpsimd.tensor_tensor(
                                pT[:, j, w0:w0 + D], pT[:, j, w0:w0 + D],
                                tril01_sb[:], mult)
                    for j in range(NTQ):
                        for (lo, hi) in chunks_for_j(j):
                            last_j = (min(NTQ, hi // D) - 1) if causal \
                                else NTQ - 1
                            nc.tensor.matmul(
                                rs[:1, lo:hi],
                                lhsT=mc(ones_sb[:]),
                                rhs=mc(pT[:, j, lo:hi]),
                                start=(j == 0),
                                stop=(j == last_j))
                    recip = spool.tile([1, T], f32, tag="recip")
                    bc = spool.tile([128, T], f32, tag="bc")
                    for (c0, c1) in ((0, 512), (512, 1024)):
                        nc.vector.reciprocal(recip[:, c0:c1], rs[:1, c0:c1])
                        nc.gpsimd.partition_broadcast(bc[:, c0:c1],
                                                      recip[:, c0:c1])
                    po = po_pool.tile([128, T], f32, tag="oT")
                    oraw = spool.tile([128, T], f32, tag="oraw")
                    for (c0, c1) in ((0, 512), (512, 1024)):
                        js = [j for j in range(NTQ)
                              if (D * j if causal else 0) < c1]
                        for j in js:
                            lo = max(D * j, c0) if causal else c0
                            nc.tensor.matmul(
                                po[:, lo:c1],
                                lhsT=mc(vT[:, j]),
                                rhs=mc(pT[:, j, lo:c1]),
                                start=(j == 0), stop=(j == js[-1]))
                        # drain PSUM promptly so the slot frees for the
                        # next (h, b) pair; scale once bc is ready
                        nc.any.tensor_copy(oraw[:, c0:c1], po[:, c0:c1])
                        nc.vector.tensor_tensor(
                            oT_sb[h][b][:, c0:c1], oraw[:, c0:c1],
                            bc[:, c0:c1], mult)

    def phase3(tc, oT_sb):
        with tc.tile_pool(name="wo", bufs=2) as wopool, \
             tc.tile_pool(name="oout", bufs=3) as opool, \
             tc.tile_pool(name="ps_out", bufs=3, space="PSUM") as pout:
            woT_r = woT_d.ap().rearrange("(ht p) e -> p ht e", p=128)
            w2_all = []
            for eh in range(2):
                esl = slice(eh * 2048, (eh + 1) * 2048)
                w2 = []
                for ht in range(QH):
                    w2t = wopool.tile([128, 2048], dt_store, tag=f"w2_{ht}",
                                      name=f"w2_{ht}_{eh}")
                    nc.sync.dma_start(w2t[:], woT_r[:, ht, esl])
                    w2.append(w2t)
                w2_all.append(w2)
            for eh in range(2):
                esl = slice(eh * 2048, (eh + 1) * 2048)
                w2 = w2_all[eh]
                for tt in range(BT // 128):
                    ot = opool.tile([128, 2048], f32, tag="ot")
                    for ecc in range(2):
                        pps = pout.tile([128, 1024], f32, tag="out",
                                        name=f"ps_out_{tt}_{ecc}")
                        for ec in range(2):
                            sl = slice(ecc * 1024 + ec * 512,
                                       ecc * 1024 + ec * 512 + 512)
                            psl = slice(ec * 512, ec * 512 + 512)
                            for h in range(QH):
                                nc.tensor.matmul(
                                    pps[:, psl],
                                    lhsT=mc(oT_sb[h][tt // NTQ][
                                        :, (tt % NTQ) * D:(tt % NTQ + 1) * D]),
                                    rhs=mc(w2[h][:, sl]),
                                    start=(h == 0), stop=(h == QH - 1))
                        osl = slice(ecc * 1024, ecc * 1024 + 1024)
                        if ecc == 0:
                            nc.vector.tensor_copy(ot[:, osl], pps[:])
                        else:
                            nc.scalar.copy(ot[:, osl], pps[:])
                    nc.sync.dma_start(
                        outp_d.ap()[tt * 128:(tt + 1) * 128, esl],
                        ot[:])

    with tile.TileContext(nc) as tc:
        with tc.tile_pool(name="consts", bufs=1) as consts:
            cosT_sb = consts.tile([D, T], f32)
            sinT_sb = consts.tile([D, T], f32)
            PT_sb = consts.tile([D, D], dt_store)
            maskT_sb = consts.tile([D, D], f32)
            tril01_sb = consts.tile([D, D], dt_store)
            ones_sb = consts.tile([D, 1], dt_store)
            ident_sb = consts.tile([D, D], dt_store)

            def issue_consts():
                nc.sync.dma_start(cosT_sb[:], cosT_d.ap()[:])
                nc.sync.dma_start(sinT_sb[:], sinT_d.ap()[:])
                nc.sync.dma_start(PT_sb[:], PT_d.ap()[:])
                nc.sync.dma_start(maskT_sb[:], maskT_d.ap()[:])
                nc.sync.dma_start(tril01_sb[:], tril01_d.ap()[:])
                nc.sync.dma_start(ones_sb[:], ones1_d.ap()[:])
                nc.sync.dma_start(ident_sb[:], ident_d.ap()[:])

            with tc.tile_pool(name="persist", bufs=1) as persist:
                qkvT_sb = [[persist.tile([128, T], dt_store,
                                         tag=f"qkv_{ft}_{bb}",
                                         name=f"qkvT_{ft}_{bb}")
                            for bb in range(B)] for ft in range(FT)]
                phase1(tc, qkvT_sb, issue_consts)
                with tc.tile_pool(name="persist2", bufs=1) as persist2:
                    oT_sb = [[persist2.tile([128, T], dt_store,
                                            tag=f"oT_{hh}_{bb}",
                                            name=f"oT_{hh}_{bb}")
                              for bb in range(B)] for hh in range(QH)]
                    phase2(tc, qkvT_sb, oT_sb)
                    phase3(tc, oT_sb)

    nc.compile()
    return nc


# ---------------------------------------------------------------- runner

class _Runner:
    """Compile once, keep a no-donation jitted SPMD callable."""

    def __init__(self, causal=True):
        import jax
        from jax.sharding import Mesh, PartitionSpec
        try:
            from jax.experimental.shard_map import shard_map
        except ImportError:  # newer jax
            from jax.sharding import shard_map
        from concourse import mybir
        from concourse.bass2jax import (_bass_exec_p, install_neuronx_cc_hook,
                                        partition_id_tensor)

        self.jax = jax
        self.nc = _build_nc(causal=causal)
        nc = self.nc
        install_neuronx_cc_hook()

        partition_name = (nc.partition_id_tensor.name
                          if nc.partition_id_tensor else None)
        in_names, out_names, out_avals, zero_outs = [], [], [], []
        for alloc in nc.m.functions[0].allocations:
            if not isinstance(alloc, mybir.MemoryLocationSet):
                continue
            name = alloc.memorylocations[0].name
            if alloc.kind == "ExternalInput":
                if name != partition_name:
                    in_names.append(name)
            elif alloc.kind == "ExternalOutput":
                out_names.append(name)
                shape = tuple(alloc.tensor_shape)
                dtype = mybir.dt.np(alloc.dtype)
                out_avals.append(jax.core.ShapedArray(shape, dtype))
                zero_outs.append(np.zeros(shape, dtype))
        self.in_names, self.out_names = in_names, out_names
        self.zero_outs = zero_outs
        n_params = len(in_names)
        in_names_all = list(in_names) + list(out_names)
        if partition_name is not None:
            in_names_all.append(partition_name)

        def _body(*args):
            operands = list(args)
            if partition_name is not None:
                operands.append(partition_id_tensor())
            outs = _bass_exec_p.bind(
                *operands, out_avals=tuple(out_avals),
                in_names=tuple(in_names_all), out_names=tuple(out_names),
                lowering_input_output_aliases=(),
                sim_require_finite=True, sim_require_nnan=True, nc=nc)
            return tuple(outs)

        devices = jax.devices()[:M]
        assert len(devices) == M, f"need {M} cores, found {len(jax.devices())}"
        mesh = Mesh(np.asarray(devices), ("core",))
        self.mesh = mesh
        in_specs = (PartitionSpec("core"),) * (n_params + len(out_names))
        out_specs = (PartitionSpec("core"),) * len(out_names)
        # Donate the output-shaped args: the NEFF fully overwrites every
        # output tensor, so we ping-pong the previous call's outputs in as
        # the next call's donated output buffers.
        donate = tuple(range(n_params, n_params + len(out_names)))
        self.fn = jax.jit(
            shard_map(_body, mesh=mesh, in_specs=in_specs,
                      out_specs=out_specs, check_rep=False),
            keep_unused=True, donate_argnums=donate)

    def put_args(self, in_maps):
        jax = self.jax
        from jax.sharding import NamedSharding, PartitionSpec
        sh = NamedSharding(self.mesh, PartitionSpec("core"))
        concat_in = [np.concatenate([in_maps[c][nm] for c in range(M)], axis=0)
                     for nm in self.in_names]
        args = [jax.device_put(x, sh) for x in concat_in]
        self._outbufs = [
            jax.device_put(np.zeros((M * z.shape[0], *z.shape[1:]), z.dtype), sh)
            for z in self.zero_outs]
        return args

    def run(self, args):
        outs = self.fn(*args, *self._outbufs)
        self.jax.block_until_ready(outs)
        self._outbufs = list(outs)   # donated ping-pong
        return outs

    def gather(self, outs):
        """Sum the 8 partials of 'outp' -> full [B,T,H,D] output."""
        i = self.out_names.index("outp")
        arr = np.asarray(outs[i]).reshape(M, BT, CD)
        return arr.sum(0, dtype=np.float32).reshape(B, T, H, D)


_RUNNERS = {}


def _get_runner(causal=True):
    if causal not in _RUNNERS:
        _RUNNERS[causal] = _Runner(causal=causal)
    return _RUNNERS[causal]


def _mask_kind(mask_w, mask_b):
    tril = np.tril(np.ones((T, T), np.float32))
    if (np.array_equal(mask_w[0, 0], tril)
            and np.allclose(mask_b[0, 0], (1.0 - tril) * NEG)):
        return "causal"
    if (mask_w == 1.0).all() and (mask_b == 0.0).all():
        return "allpass"
    return "other"


def _numpy_fallback(stm, Wq, Wk, Wv, Wo, cos, sin, mask_w, mask_b):
    x = stm.reshape(B, T, H * D).astype(np.float32)
    q = (x @ Wq.T).reshape(B, T, H, D)
    k = (x @ Wk.T).reshape(B, T, KV, D)
    v = (x @ Wv.T).reshape(B, T, KV, D)
    k = np.repeat(k, H // KV, axis=2)
    v = np.repeat(v, H // KV, axis=2)

    def rope(t):
        half = D // 2
        t2 = np.concatenate([-t[..., half:], t[..., :half]], -1)
        return t * cos + t2 * sin

    q, k = rope(q), rope(k)
    attn = np.einsum("bqhd,bkhd->bhqk", q, k).astype(np.float32) * SCALE
    attn = attn * mask_w + mask_b
    attn = attn - attn.max(-1, keepdims=True)
    attn = np.exp(attn)
    attn = attn / attn.sum(-1, keepdims=True)
    o = np.einsum("bhqk,bkhd->bqhd", attn, v).astype(np.float32)
    return (o.reshape(B, T, H * D) @ Wo.T).reshape(B, T, H, D)


def kernel(stm, Wq, Wk, Wv, Wo, cos, sin, mask_w, mask_b):
    stm = np.asarray(stm, np.float32)
    Wq, Wk, Wv, Wo = (np.asarray(a, np.float32) for a in (Wq, Wk, Wv, Wo))
    cos, sin = np.asarray(cos, np.float32), np.asarray(sin, np.float32)
    mask_w, mask_b = (np.asarray(a, np.float32) for a in (mask_w, mask_b))

    kind = _mask_kind(mask_w, mask_b)
    if kind == "other":
        return _numpy_fallback(stm, Wq, Wk, Wv, Wo, cos, sin, mask_w, mask_b)

    runner = _get_runner(causal=(kind == "causal"))
    in_maps = host_prep(stm, Wq, Wk, Wv, Wo, cos, sin, mask_b)
    args = runner.put_args(in_maps)
    outs = runner.run(args)
    return runner.gather(outs)
